# revision 13
# baseline (speedup 1.0000x reference)
"""Trainium2 Bass kernel for the BaseMemory coref scoring module.

Computes, for full inputs (M=65536 memory slots, D=768, E=20, H=64):
    score = relu(pair @ W1 + b1) @ W2 + b2, masked with ent_counter>0,
    where pair = [mem, ment, mem*ment, dist_emb, cnt_emb].

Sharding: data-parallel over the cluster dimension M across 8 NeuronCores.
Each core's shard of mem_vectors is laid out [D, MS] (contraction-major) so
the PE consumes it directly; all FLOPs and all HBM traffic stay on device.

Key folds (host side, O(D*H) + O(M) work on the small tensors only):
  - mem@W1_mem + (mem*ment)@W1_had = mem @ (W1_mem + diag(ment)@W1_had)
  - ment@W1_ment + b1 folded into the 10-row dist bucket table
  - bucket one-hots precomputed on host (O(M) int compares) and streamed
    as a [22, MS] bf16 plane; contracted on the PE against the folded
    10-row tables (masking folded into the PE accumulation, exact)
  - mem_vectors streamed as bf16: halves HBM traffic (the roofline term);
    all accumulation stays fp32 in PSUM
"""

import os
import numpy as np

# The bass kernel executes through the axon PJRT backend; make sure jax can
# see it even if the caller pinned JAX_PLATFORMS (e.g. to "cpu").
_jp = os.environ.get("JAX_PLATFORMS")
if _jp is not None and _jp != "" and "axon" not in _jp:
    os.environ["JAX_PLATFORMS"] = "axon," + _jp

M, D, E, H = 65536, 768, 20, 64
N_CORES = 8
MS = M // N_CORES          # rows per core = 8192
GROUP = 512                # rows per PE matmul group
N_GROUPS = MS // GROUP     # 16
SG = 4                     # groups per DMA super-group
N_SG = N_GROUPS // SG      # 4
KCH = D // 128             # 6 contraction chunks
NF = 22                    # 10 dist onehot, 10 cnt onehot, notmask, ones
HT = H + NF                # 86 rows of the score-matmul rhs
BIG = float(2 ** 14)       # pre-relu kill value for masked rows (fp16-exact)

_CACHE = {}


def _build():
    """Build + compile the 8-core SPMD bass program once per process."""
    if "nc" in _CACHE:
        return _CACHE["nc"]

    import concourse.bass as bass
    import concourse.mybir as mybir
    import concourse.tile as tile
    from concourse import bacc

    F32 = mybir.dt.float32
    BF16 = mybir.dt.bfloat16
    FP16 = mybir.dt.float16

    nc = bacc.Bacc("TRN2", target_bir_lowering=False, debug=False,
                   enable_asserts=False, num_devices=N_CORES)

    xt_d = nc.dram_tensor("xt", [D, MS], BF16, kind="ExternalInput").ap()
    oh_d = nc.dram_tensor("oh", [NF, MS], FP16, kind="ExternalInput").ap()
    w1_d = nc.dram_tensor("w1", [D, H], BF16, kind="ExternalInput").ap()
    tcat_d = nc.dram_tensor("tcat", [NF, H], FP16, kind="ExternalInput").ap()
    wsc_d = nc.dram_tensor("wsc", [HT, 1], FP16, kind="ExternalInput").ap()
    out_d = nc.dram_tensor("out", [MS], F32, kind="ExternalOutput").ap()

    # xt[d, m]: tile (k, s) = [128, SG*GROUP] at rows 128k, cols 2048s
    xt_r = xt_d.rearrange("(k p) (s c) -> p k s c", p=128, s=N_SG)
    w1_r = w1_d.rearrange("(k p) n -> p k n", p=128)    # [128, 6, 64]
    out_r = out_d.rearrange("(s c) -> s c", s=N_SG)     # [4, 2048]

    relu = mybir.ActivationFunctionType.Relu

    add = mybir.AluOpType.add

    with tile.TileContext(nc) as tc:
        with (
            tc.tile_pool(name="consts", bufs=1) as cpool,
            tc.tile_pool(name="xin", bufs=3) as px,
            tc.tile_pool(name="zs", bufs=3) as pzs,
            tc.tile_pool(name="zo", bufs=3) as pzo,
            tc.tile_pool(name="osb", bufs=2) as posb,
            tc.tile_pool(name="psz", bufs=4, space="PSUM") as psz,
            tc.tile_pool(name="pss", bufs=2, space="PSUM") as pss,
        ):
            # consts issue on the scalar HWDGE queue so the big xt DMAs
            # (sync queue) start immediately
            w1t = cpool.tile([128, KCH, H], BF16, tag="w1t")
            nc.scalar.dma_start(w1t[:], w1_r[:])
            # tcat lives at base partition 64 so lhsT/rhs base partitions
            # match in the feature-accumulation matmul
            tcat_full = cpool.tile([HT, H], FP16, tag="tcat")
            tcat = tcat_full[H:HT, :]
            nc.scalar.dma_start(tcat, tcat_d[:])
            wsc = cpool.tile([HT, 1], FP16, tag="wsc")
            nc.scalar.dma_start(wsc[:], wsc_d[:])

            # ht holds the full per-core score-matmul rhs for all groups:
            # rows 0..63 = relu(z) written per group, rows 64..85 = the
            # host-computed one-hot/mask plane, DMA'd once.
            ht = cpool.tile([HT, MS], FP16, tag="ht")
            nc.scalar.dma_start(ht[H:HT, :], oh_d[:])

            osb_tiles = {}
            from collections import deque
            pending = deque()

            def emit_score(g):
                sc = pss.tile([1, GROUP], F32, tag="pss")
                goff = GROUP * g
                nc.tensor.matmul(sc[:], wsc[:], ht[:, goff:goff + GROUP],
                                 start=True, stop=True)
                sq = g // SG
                if g % SG == 0:
                    osb_t = posb.tile([1, SG * GROUP], F32, tag="osb")
                    osb_tiles[sq] = osb_t
                orow = osb_tiles[sq][0:1, GROUP * (g % SG):GROUP * (g % SG + 1)]
                if g % 2 == 0:
                    nc.scalar.copy(orow, sc[:])
                else:
                    nc.vector.tensor_copy(orow, sc[:])
                if g % SG == SG - 1:
                    nc.sync.dma_start(out_r[sq:sq + 1, :],
                                      osb_tiles.pop(sq)[:])

            def load_sg(s):
                # one 3MB transfer per super-group, alternating between the
                # two HWDGE rings; sg0 split so group 0's chunks land first
                xk = px.tile([128, KCH, SG * GROUP], BF16, tag="xin")
                q = nc.sync if s % 2 == 0 else nc.scalar
                if s == 0:
                    q.dma_start(xk[:, :, 0:GROUP], xt_r[:, :, s, 0:GROUP])
                    q.dma_start(xk[:, :, GROUP:], xt_r[:, :, s, GROUP:])
                else:
                    q.dma_start(xk[:], xt_r[:, :, s, :])
                return xk

            sg_tiles = {0: load_sg(0), 1: load_sg(1)}
            for s in range(N_SG):
                if s + 2 < N_SG:
                    sg_tiles[s + 2] = load_sg(s + 2)
                xk = sg_tiles.pop(s)
                for gi in range(SG):
                    g = SG * s + gi
                    off = GROUP * gi
                    goff = GROUP * g
                    # score for g-2 issues here so the PE never waits on
                    # the ACT relu of the group it just accumulated
                    if len(pending) >= 2:
                        emit_score(pending.popleft())

                    # z split over two concurrent PE column groups:
                    # even K-chunks -> zt[0:64], odd -> zt[64:128]
                    zt = psz.tile([2 * H, GROUP], F32, tag="psz")
                    for k in range(KCH):
                        half = (k % 2) * H
                        # even chain stops at the tcat matmul below; odd
                        # chain stops at its last chunk (k == 5)
                        nc.tensor.matmul(zt[half:half + H, :], w1t[:, k, :],
                                         xk[:, k, off:off + GROUP],
                                         start=(k < 2), stop=(k == KCH - 1),
                                         skip_group_check=True)
                    # feature/bias/mask contribution via the one-hot plane
                    nc.tensor.matmul(zt[0:H, :], tcat,
                                     ht[H:HT, goff:goff + GROUP],
                                     start=False, stop=True,
                                     skip_group_check=True)

                    # one instruction may read only one PSUM operand: copy
                    # the odd half to SBUF, add, then relu (engines split
                    # so DVE and ACT each carry ~1.5 ops per group)
                    zodd = pzo.tile([H, GROUP], F32, tag="zo")
                    if g % 2 == 0:
                        nc.vector.tensor_copy(zodd[:], zt[H:2 * H, :])
                    else:
                        nc.scalar.copy(zodd[:], zt[H:2 * H, :])
                    zsum = pzs.tile([H, GROUP], F32, tag="zs")
                    nc.vector.tensor_tensor(zsum[:], zt[0:H, :], zodd[:], add)
                    nc.scalar.activation(ht[0:H, goff:goff + GROUP], zsum[:],
                                         relu)
                    pending.append(g)
            while pending:
                emit_score(pending.popleft())

    nc.compile()
    _CACHE["nc"] = nc
    return nc


def _bucket(c):
    """Reference get_bucket, replicated with the same XLA CPU float ops so
    boundary cases (c = 8, 16, 32) bucket identically."""
    import math
    import jax
    import jax.numpy as jnp
    cpu = jax.devices("cpu")[0]
    with jax.default_device(cpu):
        c = jnp.asarray(c).astype(jnp.int32)
        logspace = jnp.floor(
            jnp.log(jnp.maximum(c, 1).astype(jnp.float32)) / math.log(2)
        ).astype(jnp.int32) + 3
        idx = jnp.where(c <= 4, c, logspace)
        return np.asarray(jnp.clip(idx, 0, 9))


def _prepare_maps(ment_emb, mem_vectors, dist_table, counter_table,
                  W1, b1, W2, b2, ent_counter, last_mention_start, ment_start):
    import ml_dtypes
    f32 = np.float32
    bf16 = ml_dtypes.bfloat16
    fp16 = np.float16
    ment = np.asarray(ment_emb, f32)
    mem = np.asarray(mem_vectors, f32)
    W1 = np.asarray(W1, f32)
    ms = int(np.asarray(ment_start))

    W1m, W1r, W1h = W1[0:D], W1[D:2 * D], W1[2 * D:3 * D]
    W1d, W1c = W1[3 * D:3 * D + E], W1[3 * D + E:3 * D + 2 * E]

    w1eff = (W1m + ment[:, None] * W1h).astype(f32)              # [768, 64]
    bias_vec = (np.asarray(b1, f32) + ment @ W1r).astype(f32)    # [64]
    T_d = (np.asarray(dist_table, f32) @ W1d + bias_vec).astype(f32)
    T_c = (np.asarray(counter_table, f32) @ W1c).astype(f32)
    b2v = float(np.asarray(b2, f32).reshape(-1)[0])

    tcat = np.concatenate(
        [T_d, T_c, np.full((1, H), -BIG, f32), np.zeros((1, H), f32)], 0)
    # single score matmul: rows 0..63 act on relu(z), rows 64..85 on onehot
    wsc = np.zeros((HT, 1), f32)
    wsc[0:H, 0] = np.asarray(W2, f32).reshape(-1)
    wsc[H + 20, 0] = -10000.0 - b2v
    wsc[H + 21, 0] = b2v

    cnt_i = np.asarray(ent_counter).astype(np.int64)
    dist_i = ms - np.asarray(last_mention_start).astype(np.int64)
    bd = _bucket(dist_i)                                         # [M] in 0..9
    bc = _bucket(cnt_i)                                          # [M] in 0..9
    r = np.arange(10)
    oh = np.empty((NF, M), f32)
    oh[0:10] = (bd[None, :] == r[:, None])
    oh[10:20] = (bc[None, :] == r[:, None])
    oh[20] = (cnt_i <= 0)
    oh[21] = 1.0
    oh = oh.astype(fp16)

    w1_b = w1eff.astype(bf16)
    tcat_b = tcat.astype(fp16)
    wsc_b = wsc.astype(fp16)

    in_maps = []
    for c in range(N_CORES):
        sl = slice(c * MS, (c + 1) * MS)
        in_maps.append(dict(
            xt=np.ascontiguousarray(mem[sl].T.astype(bf16)),
            oh=np.ascontiguousarray(oh[:, sl]),
            w1=w1_b, tcat=tcat_b, wsc=wsc_b))
    return in_maps


def _postprocess(results):
    out = np.empty(M + 1, np.float32)
    for c in range(N_CORES):
        out[c * MS:(c + 1) * MS] = results[c]["out"]
    out[M] = 0.0
    return out


def run_spmd(in_maps, trace=False):
    from concourse.bass_utils import run_bass_kernel_spmd
    nc = _build()
    return run_bass_kernel_spmd(nc, in_maps, list(range(N_CORES)), trace=trace)


def kernel(**inputs):
    in_maps = _prepare_maps(**inputs)
    res = run_spmd(in_maps, trace=False)
    return _postprocess(res.results)


# revision 16
# speedup vs baseline: 1.2759x; 1.2759x over previous
"""Trainium2 Bass kernel for the BaseMemory coref scoring module.

Computes, for full inputs (M=65536 memory slots, D=768, E=20, H=64):
    score = relu(pair @ W1 + b1) @ W2 + b2, masked with ent_counter>0,
    where pair = [mem, ment, mem*ment, dist_emb, cnt_emb].

Sharding: data-parallel over the cluster dimension M across 8 NeuronCores.
Each core's shard of mem_vectors is laid out [D, MS] (contraction-major) so
the PE consumes it directly; all FLOPs and all HBM traffic stay on device.

Key folds (host side, O(D*H) + O(M) work on the small tensors only):
  - mem@W1_mem + (mem*ment)@W1_had = mem @ (W1_mem + diag(ment)@W1_had)
  - ment@W1_ment + b1 folded into the 10-row dist bucket table
  - bucket one-hots precomputed on host (O(M) int compares) and streamed
    as a [22, MS] bf16 plane; contracted on the PE against the folded
    10-row tables (masking folded into the PE accumulation, exact)
  - mem_vectors streamed as bf16: halves HBM traffic (the roofline term);
    all accumulation stays fp32 in PSUM
"""

import os
import numpy as np

# The bass kernel executes through the axon PJRT backend; make sure jax can
# see it even if the caller pinned JAX_PLATFORMS (e.g. to "cpu").
_jp = os.environ.get("JAX_PLATFORMS")
if _jp is not None and _jp != "" and "axon" not in _jp:
    os.environ["JAX_PLATFORMS"] = "axon," + _jp

M, D, E, H = 65536, 768, 20, 64
N_CORES = 8
MS = M // N_CORES          # rows per core = 8192
GROUP = 512                # rows per PE matmul group
N_GROUPS = MS // GROUP     # 16
SG = 4                     # groups per DMA super-group
N_SG = N_GROUPS // SG      # 4
KCH = D // 128             # 6 contraction chunks
NF = 22                    # 10 dist onehot, 10 cnt onehot, notmask, ones
HT = H + NF                # 86 rows of the score-matmul rhs
BIG = float(2 ** 14)       # pre-relu kill value for masked rows (fp16-exact)

_CACHE = {}


def _build():
    """Build + compile the 8-core SPMD bass program once per process."""
    if "nc" in _CACHE:
        return _CACHE["nc"]

    import concourse.bass as bass
    import concourse.mybir as mybir
    import concourse.tile as tile
    from concourse import bacc

    F32 = mybir.dt.float32
    BF16 = mybir.dt.bfloat16
    FP16 = mybir.dt.float16

    nc = bacc.Bacc("TRN2", target_bir_lowering=False, debug=False,
                   enable_asserts=False, num_devices=N_CORES)

    xt_d = nc.dram_tensor("xt", [D, MS], BF16, kind="ExternalInput").ap()
    oh_d = nc.dram_tensor("oh", [NF, MS], FP16, kind="ExternalInput").ap()
    w1_d = nc.dram_tensor("w1", [D, H], BF16, kind="ExternalInput").ap()
    tcat_d = nc.dram_tensor("tcat", [NF, H], FP16, kind="ExternalInput").ap()
    wsc_d = nc.dram_tensor("wsc", [HT, 1], FP16, kind="ExternalInput").ap()
    out_d = nc.dram_tensor("out", [MS], F32, kind="ExternalOutput").ap()

    # xt[d, m]: tile (k, s) = [128, SG*GROUP] at rows 128k, cols 2048s
    xt_r = xt_d.rearrange("(k p) (s c) -> p k s c", p=128, s=N_SG)
    w1_r = w1_d.rearrange("(k p) n -> p k n", p=128)    # [128, 6, 64]
    out_r = out_d.rearrange("(s c) -> s c", s=N_SG)     # [4, 2048]

    relu = mybir.ActivationFunctionType.Relu

    with tile.TileContext(nc) as tc:
        with (
            tc.tile_pool(name="consts", bufs=1) as cpool,
            tc.tile_pool(name="xin", bufs=4) as px,
            tc.tile_pool(name="osb", bufs=2) as posb,
            tc.tile_pool(name="psz", bufs=4, space="PSUM") as psz,
            tc.tile_pool(name="pss", bufs=2, space="PSUM") as pss,
        ):
            # consts issue on the scalar HWDGE queue so the big xt DMAs
            # (sync queue) start immediately
            w1t = cpool.tile([128, KCH, H], BF16, tag="w1t")
            nc.scalar.dma_start(w1t[:], w1_r[:])
            # tcat lives at base partition 64 so lhsT/rhs base partitions
            # match in the feature-accumulation matmul
            tcat_full = cpool.tile([HT, H], FP16, tag="tcat")
            tcat = tcat_full[H:HT, :]
            nc.scalar.dma_start(tcat, tcat_d[:])
            wsc = cpool.tile([HT, 1], FP16, tag="wsc")
            nc.scalar.dma_start(wsc[:], wsc_d[:])

            # ht holds the full per-core score-matmul rhs for all groups:
            # rows 0..63 = relu(z) written per group, rows 64..85 = the
            # host-computed one-hot/mask plane, DMA'd once.
            ht = cpool.tile([HT, MS], FP16, tag="ht")
            nc.scalar.dma_start(ht[H:HT, :], oh_d[:])

            osb_tiles = {}
            from collections import deque
            pending = deque()

            def emit_score(g):
                sc = pss.tile([1, GROUP], F32, tag="pss")
                goff = GROUP * g
                nc.tensor.matmul(sc[:], wsc[:], ht[:, goff:goff + GROUP],
                                 start=True, stop=True)
                sq = g // SG
                if g % SG == 0:
                    osb_t = posb.tile([1, SG * GROUP], F32, tag="osb")
                    osb_tiles[sq] = osb_t
                orow = osb_tiles[sq][0:1, GROUP * (g % SG):GROUP * (g % SG + 1)]
                if g % 2 == 0:
                    nc.scalar.copy(orow, sc[:])
                else:
                    nc.vector.tensor_copy(orow, sc[:])
                if g % SG == SG - 1:
                    nc.sync.dma_start(out_r[sq:sq + 1, :],
                                      osb_tiles.pop(sq)[:])

            def load_sg(s):
                # one 3MB transfer per super-group, alternating between the
                # two HWDGE rings; sg0 split so group 0's chunks land first
                xk = px.tile([128, KCH, SG * GROUP], BF16, tag="xin")
                q = nc.sync if s % 2 == 0 else nc.scalar
                if s == 0:
                    q.dma_start(xk[:, :, 0:GROUP], xt_r[:, :, s, 0:GROUP])
                    q.dma_start(xk[:, :, GROUP:], xt_r[:, :, s, GROUP:])
                else:
                    q.dma_start(xk[:], xt_r[:, :, s, :])
                return xk

            # all four super-group loads issue upfront: DMA streams at full
            # rate with no dependence on compute progress
            sg_tiles = {s: load_sg(s) for s in range(N_SG)}
            for s in range(N_SG):
                xk = sg_tiles.pop(s)
                for pi in range(SG // 2):
                    # two groups per PSUM tile, computed on the two PE
                    # column halves concurrently (tile_position inferred
                    # from the output base partition)
                    g0 = SG * s + 2 * pi
                    g1 = g0 + 1
                    off0 = GROUP * 2 * pi
                    off1 = off0 + GROUP
                    go0 = GROUP * g0
                    go1 = GROUP * g1
                    # scores trail ~2 pairs so the PE never waits on relu
                    while len(pending) >= 3:
                        emit_score(pending.popleft())

                    zt = psz.tile([2 * H, GROUP], F32, tag="psz")
                    for k in range(KCH):
                        nc.tensor.matmul(zt[0:H, :], w1t[:, k, :],
                                         xk[:, k, off0:off0 + GROUP],
                                         start=(k == 0), stop=False,
                                         skip_group_check=True)
                        nc.tensor.matmul(zt[H:2 * H, :], w1t[:, k, :],
                                         xk[:, k, off1:off1 + GROUP],
                                         start=(k == 0), stop=False,
                                         skip_group_check=True)
                    # feature/bias/mask contribution via the one-hot plane
                    nc.tensor.matmul(zt[0:H, :], tcat,
                                     ht[H:HT, go0:go0 + GROUP],
                                     start=False, stop=True,
                                     skip_group_check=True)
                    nc.tensor.matmul(zt[H:2 * H, :], tcat,
                                     ht[H:HT, go1:go1 + GROUP],
                                     start=False, stop=True,
                                     skip_group_check=True)

                    nc.scalar.activation(ht[0:H, go0:go0 + GROUP],
                                         zt[0:H, :], relu)
                    nc.vector.tensor_scalar_max(ht[0:H, go1:go1 + GROUP],
                                                zt[H:2 * H, :], 0.0)
                    pending.append(g0)
                    pending.append(g1)
            while pending:
                emit_score(pending.popleft())

    nc.compile()
    _CACHE["nc"] = nc
    return nc


def _bucket(c):
    """Reference get_bucket, replicated with the same XLA CPU float ops so
    boundary cases (c = 8, 16, 32) bucket identically."""
    import math
    import jax
    import jax.numpy as jnp
    cpu = jax.devices("cpu")[0]
    with jax.default_device(cpu):
        c = jnp.asarray(c).astype(jnp.int32)
        logspace = jnp.floor(
            jnp.log(jnp.maximum(c, 1).astype(jnp.float32)) / math.log(2)
        ).astype(jnp.int32) + 3
        idx = jnp.where(c <= 4, c, logspace)
        return np.asarray(jnp.clip(idx, 0, 9))


def _prepare_maps(ment_emb, mem_vectors, dist_table, counter_table,
                  W1, b1, W2, b2, ent_counter, last_mention_start, ment_start):
    import ml_dtypes
    f32 = np.float32
    bf16 = ml_dtypes.bfloat16
    fp16 = np.float16
    ment = np.asarray(ment_emb, f32)
    mem = np.asarray(mem_vectors, f32)
    W1 = np.asarray(W1, f32)
    ms = int(np.asarray(ment_start))

    W1m, W1r, W1h = W1[0:D], W1[D:2 * D], W1[2 * D:3 * D]
    W1d, W1c = W1[3 * D:3 * D + E], W1[3 * D + E:3 * D + 2 * E]

    w1eff = (W1m + ment[:, None] * W1h).astype(f32)              # [768, 64]
    bias_vec = (np.asarray(b1, f32) + ment @ W1r).astype(f32)    # [64]
    T_d = (np.asarray(dist_table, f32) @ W1d + bias_vec).astype(f32)
    T_c = (np.asarray(counter_table, f32) @ W1c).astype(f32)
    b2v = float(np.asarray(b2, f32).reshape(-1)[0])

    tcat = np.concatenate(
        [T_d, T_c, np.full((1, H), -BIG, f32), np.zeros((1, H), f32)], 0)
    # single score matmul: rows 0..63 act on relu(z), rows 64..85 on onehot
    wsc = np.zeros((HT, 1), f32)
    wsc[0:H, 0] = np.asarray(W2, f32).reshape(-1)
    wsc[H + 20, 0] = -10000.0 - b2v
    wsc[H + 21, 0] = b2v

    cnt_i = np.asarray(ent_counter).astype(np.int64)
    dist_i = ms - np.asarray(last_mention_start).astype(np.int64)
    bd = _bucket(dist_i)                                         # [M] in 0..9
    bc = _bucket(cnt_i)                                          # [M] in 0..9
    r = np.arange(10)
    oh = np.empty((NF, M), f32)
    oh[0:10] = (bd[None, :] == r[:, None])
    oh[10:20] = (bc[None, :] == r[:, None])
    oh[20] = (cnt_i <= 0)
    oh[21] = 1.0
    oh = oh.astype(fp16)

    w1_b = w1eff.astype(bf16)
    tcat_b = tcat.astype(fp16)
    wsc_b = wsc.astype(fp16)

    in_maps = []
    for c in range(N_CORES):
        sl = slice(c * MS, (c + 1) * MS)
        in_maps.append(dict(
            xt=np.ascontiguousarray(mem[sl].T.astype(bf16)),
            oh=np.ascontiguousarray(oh[:, sl]),
            w1=w1_b, tcat=tcat_b, wsc=wsc_b))
    return in_maps


def _postprocess(results):
    out = np.empty(M + 1, np.float32)
    for c in range(N_CORES):
        out[c * MS:(c + 1) * MS] = results[c]["out"]
    out[M] = 0.0
    return out


def run_spmd(in_maps, trace=False):
    from concourse.bass_utils import run_bass_kernel_spmd
    nc = _build()
    return run_bass_kernel_spmd(nc, in_maps, list(range(N_CORES)), trace=trace)


def kernel(**inputs):
    in_maps = _prepare_maps(**inputs)
    res = run_spmd(in_maps, trace=False)
    return _postprocess(res.results)


# revision 21
# speedup vs baseline: 1.5443x; 1.2104x over previous
"""Trainium2 Bass kernel for the BaseMemory coref scoring module.

Computes, for full inputs (M=65536 memory slots, D=768, E=20, H=64):
    score = relu(pair @ W1 + b1) @ W2 + b2, masked with ent_counter>0,
    where pair = [mem, ment, mem*ment, dist_emb, cnt_emb].

Sharding: data-parallel over the cluster dimension M across 8 NeuronCores.
Each core's shard of mem_vectors is laid out [D, MS] (contraction-major) so
the PE consumes it directly; all FLOPs and all HBM traffic stay on device.

Key folds (host side, O(D*H) + O(M) work on the small tensors only):
  - mem@W1_mem + (mem*ment)@W1_had = mem @ (W1_mem + diag(ment)@W1_had)
  - ment@W1_ment + b1 folded into the 10-row dist bucket table
  - bucket one-hots precomputed on host (O(M) int compares) and streamed
    as a [22, MS] bf16 plane; contracted on the PE against the folded
    10-row tables (masking folded into the PE accumulation, exact)
  - mem_vectors streamed as bf16: halves HBM traffic (the roofline term);
    all accumulation stays fp32 in PSUM
"""

import os
import numpy as np

# The bass kernel executes through the axon PJRT backend; make sure jax can
# see it even if the caller pinned JAX_PLATFORMS (e.g. to "cpu").
_jp = os.environ.get("JAX_PLATFORMS")
if _jp is not None and _jp != "" and "axon" not in _jp:
    os.environ["JAX_PLATFORMS"] = "axon," + _jp

M, D, E, H = 65536, 768, 20, 64
N_CORES = 8
MS = M // N_CORES          # rows per core = 8192
GROUP = 512                # rows per PE matmul group
N_GROUPS = MS // GROUP     # 16
SG = 4                     # groups per DMA super-group
N_SG = N_GROUPS // SG      # 4
KCH = D // 128             # 6 contraction chunks
NF = 22                    # 10 dist onehot, 10 cnt onehot, notmask, ones
HT = H + NF                # 86 rows of the score-matmul rhs
BIG = float(2 ** 14)       # pre-relu kill value for masked rows (fp16-exact)

_CACHE = {}


def _build():
    """Build + compile the 8-core SPMD bass program once per process."""
    if "nc" in _CACHE:
        return _CACHE["nc"]

    import concourse.bass as bass
    import concourse.mybir as mybir
    import concourse.tile as tile
    from concourse import bacc

    F32 = mybir.dt.float32
    BF16 = mybir.dt.bfloat16
    FP16 = mybir.dt.float16

    nc = bacc.Bacc("TRN2", target_bir_lowering=False, debug=False,
                   enable_asserts=False, num_devices=N_CORES)

    xt_d = nc.dram_tensor("xt", [D, MS], BF16, kind="ExternalInput").ap()
    oh_d = nc.dram_tensor("oh", [NF, MS], FP16, kind="ExternalInput").ap()
    w1_d = nc.dram_tensor("w1", [D, H], BF16, kind="ExternalInput").ap()
    tcat_d = nc.dram_tensor("tcat", [NF, H], FP16, kind="ExternalInput").ap()
    wsc_d = nc.dram_tensor("wsc", [HT, 1], FP16, kind="ExternalInput").ap()
    out_d = nc.dram_tensor("out", [MS], F32, kind="ExternalOutput").ap()

    # xt[d, m]: tile (k, s) = [128, SG*GROUP] at rows 128k, cols 2048s
    xt_r2 = xt_d.rearrange("(kp k2 p) (s c) -> p kp k2 s c", p=128, k2=2,
                           s=N_SG)
    w1_r = w1_d.rearrange("(k p) n -> p k n", p=128)    # [128, 6, 64]
    out_r = out_d.rearrange("(s c) -> s c", s=N_SG)     # [4, 2048]

    relu = mybir.ActivationFunctionType.Relu

    with tile.TileContext(nc) as tc:
        with (
            tc.tile_pool(name="consts", bufs=1) as cpool,
            tc.tile_pool(name="xin", bufs=12) as px,
            tc.tile_pool(name="osb", bufs=2) as posb,
            tc.tile_pool(name="psz", bufs=4, space="PSUM") as psz,
            tc.tile_pool(name="pss", bufs=2, space="PSUM") as pss,
        ):
            # consts issue on the scalar HWDGE queue so the big xt DMAs
            # (sync queue) start immediately
            w1t = cpool.tile([128, KCH, H], BF16, tag="w1t")
            nc.scalar.dma_start(w1t[:], w1_r[:])
            # tcat lives at base partition 64 so lhsT/rhs base partitions
            # match in the feature-accumulation matmul
            tcat_full = cpool.tile([HT, H], FP16, tag="tcat")
            tcat = tcat_full[H:HT, :]
            nc.scalar.dma_start(tcat, tcat_d[:])
            wsc = cpool.tile([HT, 1], FP16, tag="wsc")
            nc.scalar.dma_start(wsc[:], wsc_d[:])

            # ht holds the full per-core score-matmul rhs for all groups:
            # rows 0..63 = relu(z) written per group, rows 64..85 = the
            # host-computed one-hot/mask plane, DMA'd once.
            ht = cpool.tile([HT, MS], FP16, tag="ht")
            nc.scalar.dma_start(ht[H:HT, :], oh_d[:])

            osb_tiles = {}
            from collections import deque
            pending = deque()

            def emit_score(g):
                sc = pss.tile([1, GROUP], F32, tag="pss")
                goff = GROUP * g
                nc.tensor.matmul(sc[:], wsc[:], ht[:, goff:goff + GROUP],
                                 start=True, stop=True)
                sq = g // SG
                if g % SG == 0:
                    osb_t = posb.tile([1, SG * GROUP], F32, tag="osb")
                    osb_tiles[sq] = osb_t
                orow = osb_tiles[sq][0:1, GROUP * (g % SG):GROUP * (g % SG + 1)]
                if g % 2 == 0:
                    nc.scalar.copy(orow, sc[:])
                else:
                    nc.vector.tensor_copy(orow, sc[:])
                if g % SG == SG - 1:
                    nc.scalar.dma_start(out_r[sq:sq + 1, :],
                                        osb_tiles.pop(sq)[:])

            def load_sg(s):
                # x streams on the sync ring only, in consumption order, so
                # arrival order matches the compute schedule at full rate;
                # sg0 split at the pair boundary so pair 0 starts early
                xks = []
                for kp in range(KCH // 2):
                    xk = px.tile([128, 2, SG * GROUP], BF16, tag="xin")
                    if s == 0:
                        nc.sync.dma_start(xk[:, :, 0:2 * GROUP],
                                          xt_r2[:, kp, :, s, 0:2 * GROUP])
                        nc.sync.dma_start(xk[:, :, 2 * GROUP:],
                                          xt_r2[:, kp, :, s, 2 * GROUP:])
                    else:
                        nc.sync.dma_start(xk[:], xt_r2[:, kp, :, s, :])
                    xks.append(xk)
                return xks

            # all super-group loads issue upfront: DMA streams at full
            # rate with no dependence on compute progress
            sg_tiles = {s: load_sg(s) for s in range(N_SG)}
            for s in range(N_SG):
                xks = sg_tiles.pop(s)
                for pi in range(SG // 2):
                    # two groups per PSUM tile, computed on the two PE
                    # column halves concurrently (tile_position inferred
                    # from the output base partition)
                    g0 = SG * s + 2 * pi
                    g1 = g0 + 1
                    off0 = GROUP * 2 * pi
                    off1 = off0 + GROUP
                    go0 = GROUP * g0
                    go1 = GROUP * g1

                    zt = psz.tile([2 * H, GROUP], F32, tag="psz")
                    for k in range(KCH):
                        xkr = xks[k // 2]
                        nc.tensor.matmul(zt[0:H, :], w1t[:, k, :],
                                         xkr[:, k % 2, off0:off0 + GROUP],
                                         start=(k == 0), stop=False,
                                         skip_group_check=True)
                        nc.tensor.matmul(zt[H:2 * H, :], w1t[:, k, :],
                                         xkr[:, k % 2, off1:off1 + GROUP],
                                         start=(k == 0), stop=False,
                                         skip_group_check=True)
                    # feature/bias/mask contribution via the one-hot plane
                    nc.tensor.matmul(zt[0:H, :], tcat,
                                     ht[H:HT, go0:go0 + GROUP],
                                     start=False, stop=True,
                                     skip_group_check=True)
                    nc.tensor.matmul(zt[H:2 * H, :], tcat,
                                     ht[H:HT, go1:go1 + GROUP],
                                     start=False, stop=True,
                                     skip_group_check=True)

                    # scores for the previous pair issue behind this pair's
                    # z accumulation so the PE never waits on a relu
                    while len(pending) > 2:
                        emit_score(pending.popleft())

                    nc.scalar.activation(ht[0:H, go0:go0 + GROUP],
                                         zt[0:H, :], relu)
                    nc.vector.tensor_scalar_max(ht[0:H, go1:go1 + GROUP],
                                                zt[H:2 * H, :], 0.0)
                    pending.append(g0)
                    pending.append(g1)
            while pending:
                emit_score(pending.popleft())

    nc.compile()
    _CACHE["nc"] = nc
    return nc


def _bucket(c):
    """Reference get_bucket, replicated with the same XLA CPU float ops so
    boundary cases (c = 8, 16, 32) bucket identically."""
    import math
    import jax
    import jax.numpy as jnp
    cpu = jax.devices("cpu")[0]
    with jax.default_device(cpu):
        c = jnp.asarray(c).astype(jnp.int32)
        logspace = jnp.floor(
            jnp.log(jnp.maximum(c, 1).astype(jnp.float32)) / math.log(2)
        ).astype(jnp.int32) + 3
        idx = jnp.where(c <= 4, c, logspace)
        return np.asarray(jnp.clip(idx, 0, 9))


def _prepare_maps(ment_emb, mem_vectors, dist_table, counter_table,
                  W1, b1, W2, b2, ent_counter, last_mention_start, ment_start):
    import ml_dtypes
    f32 = np.float32
    bf16 = ml_dtypes.bfloat16
    fp16 = np.float16
    ment = np.asarray(ment_emb, f32)
    mem = np.asarray(mem_vectors, f32)
    W1 = np.asarray(W1, f32)
    ms = int(np.asarray(ment_start))

    W1m, W1r, W1h = W1[0:D], W1[D:2 * D], W1[2 * D:3 * D]
    W1d, W1c = W1[3 * D:3 * D + E], W1[3 * D + E:3 * D + 2 * E]

    w1eff = (W1m + ment[:, None] * W1h).astype(f32)              # [768, 64]
    bias_vec = (np.asarray(b1, f32) + ment @ W1r).astype(f32)    # [64]
    T_d = (np.asarray(dist_table, f32) @ W1d + bias_vec).astype(f32)
    T_c = (np.asarray(counter_table, f32) @ W1c).astype(f32)
    b2v = float(np.asarray(b2, f32).reshape(-1)[0])

    tcat = np.concatenate(
        [T_d, T_c, np.full((1, H), -BIG, f32), np.zeros((1, H), f32)], 0)
    # single score matmul: rows 0..63 act on relu(z), rows 64..85 on onehot
    wsc = np.zeros((HT, 1), f32)
    wsc[0:H, 0] = np.asarray(W2, f32).reshape(-1)
    wsc[H + 20, 0] = -10000.0 - b2v
    wsc[H + 21, 0] = b2v

    cnt_i = np.asarray(ent_counter).astype(np.int64)
    dist_i = ms - np.asarray(last_mention_start).astype(np.int64)
    bd = _bucket(dist_i)                                         # [M] in 0..9
    bc = _bucket(cnt_i)                                          # [M] in 0..9
    r = np.arange(10)
    oh = np.empty((NF, M), f32)
    oh[0:10] = (bd[None, :] == r[:, None])
    oh[10:20] = (bc[None, :] == r[:, None])
    oh[20] = (cnt_i <= 0)
    oh[21] = 1.0
    oh = oh.astype(fp16)

    w1_b = w1eff.astype(bf16)
    tcat_b = tcat.astype(fp16)
    wsc_b = wsc.astype(fp16)

    in_maps = []
    for c in range(N_CORES):
        sl = slice(c * MS, (c + 1) * MS)
        in_maps.append(dict(
            xt=np.ascontiguousarray(mem[sl].T.astype(bf16)),
            oh=np.ascontiguousarray(oh[:, sl]),
            w1=w1_b, tcat=tcat_b, wsc=wsc_b))
    return in_maps


def _postprocess(results):
    out = np.empty(M + 1, np.float32)
    for c in range(N_CORES):
        out[c * MS:(c + 1) * MS] = results[c]["out"]
    out[M] = 0.0
    return out


def run_spmd(in_maps, trace=False):
    from concourse.bass_utils import run_bass_kernel_spmd
    nc = _build()
    return run_bass_kernel_spmd(nc, in_maps, list(range(N_CORES)), trace=trace)


def kernel(**inputs):
    in_maps = _prepare_maps(**inputs)
    res = run_spmd(in_maps, trace=False)
    return _postprocess(res.results)
